# revision 6
# baseline (speedup 1.0000x reference)
"""Trainium2 Bass kernel for nn_CrossAttentionFusion.

Math: softmax over kv_len==1 is identically 1.0, so the attention output is
v broadcast over the N (patch) axis and the whole module reduces to

    out[b, n, :] = cnn[b] @ (Wkv[:, C:] @ Wp) + bp        (independent of n)

W_eff = Wkv[:, C:] @ Wp is a weight-only constant, folded on the host.

Sharding: 8 cores = 4 batch-groups x 2 column-groups. Each core computes
y = cnn_shard @ W_eff_slice + bp_slice for its 16 batches x 384 columns and
writes the [16, 576, 384] output block (14.16 MB; the kernel is bound by
this HBM write stream).

Bandwidth tricks vs the fp32 data-parallel baseline:
  * weights and activations stream in bf16 (error ~2e-3 << 2e-2 gate),
    cutting read traffic from 6.4 MB to ~2.1 MB per core;
  * the cnn shard is host-replicated 8x along the M axis of the lhsT so the
    K-chunk accumulation produces y directly REPLICATED across all 128 PSUM
    partitions (partition p holds y[p//8]) - no one-hot broadcast matmul;
  * the bias rides as a cheap bf16 K=1 accumulation chunk;
  * weights stream in 8 groups of 2 k-chunks so the PE trails the DMA by
    ~one group instead of waiting for the whole tensor (DMA completion
    sems fire near the end of the read phase when transfers are large);
  * the replicated row is materialized 4x in SBUF (bc4) so the output
    DMAs carry 6144B descriptors; the first two writes source from the
    partial bc4 prefix (1536/3072B descs) to start the write stream while
    the remaining copies land.
"""

import sys

sys.path.insert(0, "/opt/trn_rl_repo")

import ml_dtypes
import numpy as np

import concourse.bass as bass
import concourse.mybir as mybir
from concourse import bacc
from concourse.bass_utils import run_bass_kernel_spmd
from concourse.tile import TileContext

F32 = mybir.dt.float32
BF16 = mybir.dt.bfloat16
NPBF16 = np.dtype(ml_dtypes.bfloat16)

NCORES = 8
B, N, C, CNN = 64, 576, 768, 2048
BGROUPS, CGROUPS = 4, 2          # batch groups x column groups
BS = B // BGROUPS                # 16 batches per core
CW = C // CGROUPS                # 384 columns per core
KC = CNN // 128                  # 16 k-chunks
REP = 128 // BS                  # 8 partitions per batch
ROWS_PP = N // REP               # 72 output rows per partition
RPT = 8                          # rows per partition per write DMA
NWR = ROWS_PP // RPT             # 9 write DMAs
W_GROUPS = (2, 2, 4, 4, 4)       # weight k-chunks per DMA group
NCOPIES = 4                      # replicated row copies in SBUF (desc size)


def _build_bass():
    nc = bacc.Bacc(None, target_bir_lowering=False, debug=False, num_devices=NCORES)

    x_cnn = nc.declare_dram_parameter("cnnrep", [128, KC * 128], BF16, isOutput=False)
    x_weff = nc.declare_dram_parameter("weff", [128, KC * CW], BF16, isOutput=False)
    x_bias = nc.declare_dram_parameter("biaspack", [1, 128 + CW], BF16, isOutput=False)
    y = nc.declare_dram_parameter("out", [BS, N, CW], F32, isOutput=True)

    with TileContext(nc) as tc:
        with (
            tc.tile_pool(name="singles", bufs=1) as singles,
            tc.tile_pool(name="psum_y", bufs=1, space="PSUM") as psum_y,
        ):
            # PE warm-up: junk matmul on scratch data ramps the HAM di/dt
            # throttle before the latency-critical matmuls arrive.
            wu_sb = singles.tile([128, 512], F32, tag="wu_sb")
            nc.gpsimd.memset(wu_sb[:], 0.0)
            with tc.tile_pool(name="psum_w", bufs=1, space="PSUM") as psum_w:
                ps_w = psum_w.tile([8, 512], F32, tag="ps_w")
                nc.tensor.matmul(
                    ps_w[:], wu_sb[:, 0:8], wu_sb[:, :], start=True, stop=True
                )

            # ALL reads ride the sync ring, in dependency order. HWDGE is
            # FIFO per ring, so completion sems fire incrementally as the
            # stream advances (two active rings would round-robin at packet
            # granularity and push every completion to the end of the read
            # phase). The scalar ring stays empty for the earliest writes.
            bias_t = singles.tile([1, 128 + CW], BF16, tag="bias")
            nc.sync.dma_start(out=bias_t[:], in_=x_bias[:, :])
            cnn_t = singles.tile([128, KC * 128], BF16, tag="cnn")
            half = KC * 128 // 2
            nc.sync.dma_start(out=cnn_t[:, 0:half], in_=x_cnn[:, 0:half])
            nc.sync.dma_start(out=cnn_t[:, half:], in_=x_cnn[:, half:])

            weff_t = singles.tile([128, KC * CW], BF16, tag="weff")
            kc0 = 0
            for gk in W_GROUPS:
                nc.sync.dma_start(
                    out=weff_t[:, kc0 * CW : (kc0 + gk) * CW],
                    in_=x_weff[:, kc0 * CW : (kc0 + gk) * CW],
                )
                kc0 += gk

            # y replicated: ps_y[p, c] = bp[c] + sum_k cnn[p//8, k]*Weff[k, c]
            ps_y = psum_y.tile([128, CW], F32, tag="ps_y")
            nc.tensor.matmul(
                ps_y[:],
                bias_t[:, 0:128],
                bias_t[:, 128 : 128 + CW],
                start=True,
                stop=False,
            )
            for kc in range(KC):
                nc.tensor.matmul(
                    ps_y[:],
                    cnn_t[:, kc * 128 : (kc + 1) * 128],
                    weff_t[:, kc * CW : (kc + 1) * CW],
                    start=False,
                    stop=(kc == KC - 1),
                )

            # materialize NCOPIES of the row per partition for fat write descs
            bc4 = singles.tile([128, NCOPIES * CW], F32, tag="bc4")
            for j in range(NCOPIES):
                nc.vector.tensor_copy(bc4[:, j * CW : (j + 1) * CW], ps_y[:])

            # out rows n = q*72 + s for partition p = b*8 + q; each DMA
            # writes RPT consecutive rows per partition. Sources grow with
            # the bc4 prefix so early writes launch before all copies land.
            y_v = y.rearrange("b (q s) c -> (b q) s c", q=REP)
            srcs = {
                0: bc4[:, 0:CW].unsqueeze(1).broadcast_to((128, RPT, CW)),
                1: bc4[:, 0 : 2 * CW]
                .unsqueeze(1)
                .broadcast_to((128, RPT // 2, 2 * CW)),
            }
            src_full = (
                bc4[:, :]
                .unsqueeze(1)
                .broadcast_to((128, RPT // NCOPIES, NCOPIES * CW))
            )
            for i in range(NWR):
                eng = nc.scalar if i % 2 == 0 else nc.sync
                eng.dma_start(
                    out=y_v[:, i * RPT : (i + 1) * RPT, :],
                    in_=srcs.get(i, src_full),
                )

    nc.compile()
    return nc


_NC = None


def _get_nc():
    global _NC
    if _NC is None:
        _NC = _build_bass()
    return _NC


def _prepare_in_maps(image_patches, cnn_feature_vector, Wq, Wkv, Wp, bp):
    Weff = np.ascontiguousarray(Wkv[:, C:]) @ Wp  # (2048, 768) fp32
    bp = bp.astype(np.float32)

    weff_arrs = []
    bias_arrs = []
    for cg in range(CGROUPS):
        sl = slice(cg * CW, (cg + 1) * CW)
        weff_arrs.append(
            np.ascontiguousarray(
                Weff[:, sl]
                .reshape(KC, 128, CW)
                .transpose(1, 0, 2)
                .reshape(128, KC * CW)
                .astype(NPBF16)
            )
        )
        pack = np.empty((1, 128 + CW), dtype=np.float32)
        pack[0, :128] = 1.0
        pack[0, 128:] = bp[sl]
        bias_arrs.append(pack.astype(NPBF16))

    cnn_arrs = []
    for bg in range(BGROUPS):
        shard = cnn_feature_vector[bg * BS : (bg + 1) * BS]  # (16, 2048)
        rep = np.repeat(shard, REP, axis=0)  # (128, 2048), row p = batch p//8
        cnn_arrs.append(
            np.ascontiguousarray(
                rep.reshape(128, KC, 128)
                .transpose(2, 1, 0)
                .reshape(128, KC * 128)
                .astype(NPBF16)
            )
        )

    in_maps = []
    for core in range(NCORES):
        bg, cg = core // CGROUPS, core % CGROUPS
        in_maps.append(
            {
                "cnnrep": cnn_arrs[bg],
                "weff": weff_arrs[cg],
                "biaspack": bias_arrs[cg],
            }
        )
    return in_maps


def _assemble(res):
    out = np.empty((B, N, C), dtype=np.float32)
    for core in range(NCORES):
        bg, cg = core // CGROUPS, core % CGROUPS
        out[bg * BS : (bg + 1) * BS, :, cg * CW : (cg + 1) * CW] = res.results[
            core
        ]["out"]
    return out


def kernel(**inputs) -> np.ndarray:
    inputs = {k: np.asarray(v) for k, v in inputs.items()}
    nc = _get_nc()
    in_maps = _prepare_in_maps(**inputs)
    res = run_bass_kernel_spmd(nc, in_maps, core_ids=list(range(NCORES)))
    return _assemble(res)


def kernel_traced(**inputs):
    """kernel() + HW profile; returns (output, BassKernelResults)."""
    inputs = {k: np.asarray(v) for k, v in inputs.items()}
    nc = _get_nc()
    in_maps = _prepare_in_maps(**inputs)
    res = run_bass_kernel_spmd(
        nc, in_maps, core_ids=list(range(NCORES)), trace=True
    )
    return _assemble(res), res


# revision 9
# speedup vs baseline: 1.1383x; 1.1383x over previous
"""Trainium2 Bass kernel for nn_CrossAttentionFusion.

Math: softmax over kv_len==1 is identically 1.0, so the attention output is
v broadcast over the N (patch) axis and the whole module reduces to

    out[b, n, :] = cnn[b] @ (Wkv[:, C:] @ Wp) + bp        (independent of n)

W_eff = Wkv[:, C:] @ Wp is a weight-only constant, folded on the host.

Sharding: 8 cores = 4 batch-groups x 2 column-groups. Each core computes
y = cnn_shard @ W_eff_slice + bp_slice for its 16 batches x 384 columns and
writes the [16, 576, 384] output block (14.16 MB; the kernel is bound by
this HBM write stream).

Bandwidth tricks vs the fp32 data-parallel baseline:
  * weights and activations stream in bf16 (error ~2e-3 << 2e-2 gate),
    cutting read traffic from 6.4 MB to ~2.1 MB per core;
  * the cnn shard is host-replicated 8x along the M axis of the lhsT so the
    K-chunk accumulation produces y directly REPLICATED across all 128 PSUM
    partitions (partition p holds y[p//8]) - no one-hot broadcast matmul;
  * the bias rides as a cheap bf16 K=1 accumulation chunk;
  * weights stream in 8 groups of 2 k-chunks so the PE trails the DMA by
    ~one group instead of waiting for the whole tensor (DMA completion
    sems fire near the end of the read phase when transfers are large);
  * the replicated row is materialized 4x in SBUF (bc4) so the output
    DMAs carry 6144B descriptors; the first two writes source from the
    partial bc4 prefix (1536/3072B descs) to start the write stream while
    the remaining copies land.
"""

import sys

sys.path.insert(0, "/opt/trn_rl_repo")

import ml_dtypes
import numpy as np

import concourse.bass as bass
import concourse.mybir as mybir
from concourse import bacc
from concourse.bass_utils import run_bass_kernel_spmd
from concourse.tile import TileContext

F32 = mybir.dt.float32
BF16 = mybir.dt.bfloat16
NPBF16 = np.dtype(ml_dtypes.bfloat16)

NCORES = 8
B, N, C, CNN = 64, 576, 768, 2048
BGROUPS, CGROUPS = 4, 2          # batch groups x column groups
BS = B // BGROUPS                # 16 batches per core
CW = C // CGROUPS                # 384 columns per core
KC = CNN // 128                  # 16 k-chunks
REP = 128 // BS                  # 8 partitions per batch
ROWS_PP = N // REP               # 72 output rows per partition
RPT = 8                          # rows per partition per write DMA
NWR = ROWS_PP // RPT             # 9 write DMAs
W_GROUPS = (2, 2, 4, 4, 4)       # weight k-chunks per DMA group
NCOPIES = 4                      # replicated row copies in SBUF (desc size)


def _build_bass():
    nc = bacc.Bacc(None, target_bir_lowering=False, debug=False, num_devices=NCORES)

    x_cnn = nc.declare_dram_parameter("cnnrep", [128, KC * 128], BF16, isOutput=False)
    x_weff = nc.declare_dram_parameter("weff", [128, KC * CW], BF16, isOutput=False)
    x_bias = nc.declare_dram_parameter("biaspack", [1, 128 + CW], BF16, isOutput=False)
    y = nc.declare_dram_parameter("out", [BS, N, CW], F32, isOutput=True)

    with TileContext(nc) as tc:
        with (
            tc.tile_pool(name="singles", bufs=1) as singles,
            tc.tile_pool(name="psum_y", bufs=1, space="PSUM") as psum_y,
        ):
            # PE warm-up: junk matmul on scratch data ramps the HAM di/dt
            # throttle before the latency-critical matmuls arrive.
            wu_sb = singles.tile([128, 512], F32, tag="wu_sb")
            nc.gpsimd.memset(wu_sb[:], 0.0)
            with tc.tile_pool(name="psum_w", bufs=1, space="PSUM") as psum_w:
                ps_w = psum_w.tile([8, 512], F32, tag="ps_w")
                nc.tensor.matmul(
                    ps_w[:], wu_sb[:, 0:8], wu_sb[:, :], start=True, stop=True
                )

            # Reads split across both rings with the critical chain
            # front-loaded on scalar: HWDGE rings are FIFO, and each DMA
            # gets its OWN tile so a reader waits on exactly its group's
            # completion sem (a shared tile makes every reader wait for
            # every writer of that tile). Two active rings keep the SDMA
            # per-engine descriptor pipelines deep (single-ring reads run
            # ~40% slower).
            bias_t = singles.tile([1, 128 + CW], BF16, tag="bias")
            nc.scalar.dma_start(out=bias_t[:], in_=x_bias[:, :])
            half = KC * 128 // 2
            cnn_a = singles.tile([128, half], BF16, tag="cnn_a")
            cnn_b = singles.tile([128, half], BF16, tag="cnn_b")

            def cnn_chunk(kc):
                t = cnn_a if kc < KC // 2 else cnn_b
                o = kc if kc < KC // 2 else kc - KC // 2
                return t[:, o * 128 : (o + 1) * 128]

            weff_ts = []
            kc0 = 0
            for gi, gk in enumerate(W_GROUPS):
                weff_ts.append(
                    (
                        kc0,
                        singles.tile(
                            [128, gk * CW], BF16, tag=f"weff{gi}", name=f"weff{gi}"
                        ),
                    )
                )
                kc0 += gk

            def weff_chunk(kc):
                for kcs, t in reversed(weff_ts):
                    if kc >= kcs:
                        return t[:, (kc - kcs) * CW : (kc - kcs + 1) * CW]

            # scalar: critical head (cnn_a, weff g0, cnn_b, weff g1)
            nc.scalar.dma_start(out=cnn_a[:], in_=x_cnn[:, 0:half])
            nc.scalar.dma_start(
                out=weff_ts[0][1][:], in_=x_weff[:, 0 : W_GROUPS[0] * CW]
            )
            nc.scalar.dma_start(out=cnn_b[:], in_=x_cnn[:, half:])
            g1k = W_GROUPS[0]
            nc.scalar.dma_start(
                out=weff_ts[1][1][:],
                in_=x_weff[:, g1k * CW : (g1k + W_GROUPS[1]) * CW],
            )
            # sync: bulk weight groups
            kc0 = W_GROUPS[0] + W_GROUPS[1]
            for gi in range(2, len(W_GROUPS)):
                gk = W_GROUPS[gi]
                nc.sync.dma_start(
                    out=weff_ts[gi][1][:],
                    in_=x_weff[:, kc0 * CW : (kc0 + gk) * CW],
                )
                kc0 += gk

            # y replicated: ps_y[p, c] = bp[c] + sum_k cnn[p//8, k]*Weff[k, c]
            ps_y = psum_y.tile([128, CW], F32, tag="ps_y")
            nc.tensor.matmul(
                ps_y[:],
                bias_t[:, 0:128],
                bias_t[:, 128 : 128 + CW],
                start=True,
                stop=False,
            )
            for kc in range(KC):
                nc.tensor.matmul(
                    ps_y[:],
                    cnn_chunk(kc),
                    weff_chunk(kc),
                    start=False,
                    stop=(kc == KC - 1),
                )

            # materialize NCOPIES of the row per partition for fat write descs
            bc4 = singles.tile([128, NCOPIES * CW], F32, tag="bc4")
            for j in range(NCOPIES):
                nc.vector.tensor_copy(bc4[:, j * CW : (j + 1) * CW], ps_y[:])

            # out rows n = q*72 + s for partition p = b*8 + q; each DMA
            # writes RPT consecutive rows per partition. Sources grow with
            # the bc4 prefix so early writes launch before all copies land.
            y_v = y.rearrange("b (q s) c -> (b q) s c", q=REP)
            srcs = {
                0: bc4[:, 0:CW].unsqueeze(1).broadcast_to((128, RPT, CW)),
                1: bc4[:, 0 : 2 * CW]
                .unsqueeze(1)
                .broadcast_to((128, RPT // 2, 2 * CW)),
            }
            src_full = (
                bc4[:, :]
                .unsqueeze(1)
                .broadcast_to((128, RPT // NCOPIES, NCOPIES * CW))
            )
            for i in range(NWR):
                eng = nc.scalar if i % 2 == 0 else nc.sync
                eng.dma_start(
                    out=y_v[:, i * RPT : (i + 1) * RPT, :],
                    in_=srcs.get(i, src_full),
                )

    nc.compile()
    return nc


_NC = None


def _get_nc():
    global _NC
    if _NC is None:
        _NC = _build_bass()
    return _NC


def _prepare_in_maps(image_patches, cnn_feature_vector, Wq, Wkv, Wp, bp):
    Weff = np.ascontiguousarray(Wkv[:, C:]) @ Wp  # (2048, 768) fp32
    bp = bp.astype(np.float32)

    weff_arrs = []
    bias_arrs = []
    for cg in range(CGROUPS):
        sl = slice(cg * CW, (cg + 1) * CW)
        weff_arrs.append(
            np.ascontiguousarray(
                Weff[:, sl]
                .reshape(KC, 128, CW)
                .transpose(1, 0, 2)
                .reshape(128, KC * CW)
                .astype(NPBF16)
            )
        )
        pack = np.empty((1, 128 + CW), dtype=np.float32)
        pack[0, :128] = 1.0
        pack[0, 128:] = bp[sl]
        bias_arrs.append(pack.astype(NPBF16))

    cnn_arrs = []
    for bg in range(BGROUPS):
        shard = cnn_feature_vector[bg * BS : (bg + 1) * BS]  # (16, 2048)
        rep = np.repeat(shard, REP, axis=0)  # (128, 2048), row p = batch p//8
        cnn_arrs.append(
            np.ascontiguousarray(
                rep.reshape(128, KC, 128)
                .transpose(2, 1, 0)
                .reshape(128, KC * 128)
                .astype(NPBF16)
            )
        )

    in_maps = []
    for core in range(NCORES):
        bg, cg = core // CGROUPS, core % CGROUPS
        in_maps.append(
            {
                "cnnrep": cnn_arrs[bg],
                "weff": weff_arrs[cg],
                "biaspack": bias_arrs[cg],
            }
        )
    return in_maps


def _assemble(res):
    out = np.empty((B, N, C), dtype=np.float32)
    for core in range(NCORES):
        bg, cg = core // CGROUPS, core % CGROUPS
        out[bg * BS : (bg + 1) * BS, :, cg * CW : (cg + 1) * CW] = res.results[
            core
        ]["out"]
    return out


def kernel(**inputs) -> np.ndarray:
    inputs = {k: np.asarray(v) for k, v in inputs.items()}
    nc = _get_nc()
    in_maps = _prepare_in_maps(**inputs)
    res = run_bass_kernel_spmd(nc, in_maps, core_ids=list(range(NCORES)))
    return _assemble(res)


def kernel_traced(**inputs):
    """kernel() + HW profile; returns (output, BassKernelResults)."""
    inputs = {k: np.asarray(v) for k, v in inputs.items()}
    nc = _get_nc()
    in_maps = _prepare_in_maps(**inputs)
    res = run_bass_kernel_spmd(
        nc, in_maps, core_ids=list(range(NCORES)), trace=True
    )
    return _assemble(res), res
